# revision 1
# baseline (speedup 1.0000x reference)
"""Trainium2 Bass kernel for nn_Decoder (Bahdanau-attention GRU decoder).

Sharding: 8 cores = 2 streams (right/left interleaved GRU scans) x 4 batch
groups (8 batches each).  Zero collectives; each core computes its stream's
hidden states for its batches and the full generator/log-softmax for its
output rows.

Per-core phases:
  A: ctx2 = ctx @ Wih.T + bias_fold     (attention context folded into gate
     space, so the per-step attention-weighted sum becomes a small einsum)
  B: 32 GRU steps (left stream's last step is a dummy, h zeroed via alpha)
  C: out-projection + embedding projection + generator matmul + log-softmax
"""

import os
import sys

import numpy as np

for _p in ("/opt/trn_rl_repo", "/root/.axon_site/_ro/trn_rl_repo"):
    if os.path.isdir(_p) and _p not in sys.path:
        sys.path.append(_p)

import ml_dtypes  # noqa: E402

import concourse.bass as bass  # noqa: E402
import concourse.tile as tile  # noqa: E402
from concourse import bacc, masks, mybir  # noqa: E402
from concourse import bass_utils  # noqa: E402

FP32 = mybir.dt.float32
BF16 = mybir.dt.bfloat16
AF = mybir.ActivationFunctionType
ALU = mybir.AluOpType
AX = mybir.AxisListType

B, S, T = 32, 48, 64
H, C, E, V = 1024, 2048, 512, 32000
S64 = 64          # padded source length
BL = 8            # batches per core
NSTEP = 32        # GRU steps per core (left runs one dummy step)
ROWS = NSTEP * BL  # 256 output rows per core
GN = 3 * H        # 3072 gate rows
N_CORES = 8
NVC = 63          # 62 full 512-chunks + one 256 chunk of V


def _vchunk(vc):
    return 512 if vc < 62 else 256


def decoder_kernel(tc: tile.TileContext, out: bass.AP, I: dict):
    nc = tc.nc

    from contextlib import ExitStack

    with ExitStack() as ctx:
        const = ctx.enter_context(tc.tile_pool(name="const", bufs=1))
        ident = const.tile([128, 128], FP32)
        masks.make_identity(nc, ident[:])

        pers = ctx.enter_context(tc.tile_pool(name="pers", bufs=1))
        hist = pers.tile([128, NSTEP * 64], FP32)  # slot i: [:, i*64+(jt*8+b)]

        ab_stack = ExitStack()
        ctx2pool = ab_stack.enter_context(tc.tile_pool(name="ctx2p", bufs=1))
        ctx2 = ctx2pool.tile([128, 4 * GN], BF16)  # m-tile-major, (m, n)

        # ---------------- phase A: ctx2 = ctxT.T @ wihT + bias ----------------
        with tc.tile_pool(name="pa", bufs=1) as pa, \
                tc.tile_pool(name="paw", bufs=4) as paw, \
                tc.tile_pool(name="paps", bufs=2, space="PSUM") as paps:
            ctxT_sb = pa.tile([128, 16 * 512], BF16)
            nc.sync.dma_start(
                ctxT_sb[:].rearrange("p (kt r) -> p kt r", kt=16),
                I["ctxT"].rearrange("(kt p) r -> p kt r", p=128))
            bias_bc = pa.tile([128, GN], FP32)
            nc.sync.dma_start(bias_bc[:], I["biasf"][:])

            for ng in range(6):
                psums = [paps.tile([128, 512], FP32, tag=f"pa{m}", name=f"pa{ng}_{m}")
                         for m in range(4)]
                for k in range(16):
                    wt = paw.tile([128, 512], BF16, tag="wihtile")
                    nc.sync.dma_start(
                        wt[:], I["wihT"][k * 128:(k + 1) * 128, ng * 512:(ng + 1) * 512])
                    for m in range(4):
                        nc.tensor.matmul(
                            psums[m][:],
                            ctxT_sb[:, k * 512 + m * 128: k * 512 + (m + 1) * 128],
                            wt[:], start=(k == 0), stop=(k == 15))
                for m in range(4):
                    nc.vector.tensor_add(
                        ctx2[:, m * GN + ng * 512: m * GN + (ng + 1) * 512],
                        psums[m][:], bias_bc[:, ng * 512:(ng + 1) * 512])

        # ---------------- phase B residents ----------------
        resB = ab_stack.enter_context(tc.tile_pool(name="resB", bufs=1))
        wq_sb = resB.tile([128, 8 * H], BF16)
        nc.sync.dma_start(wq_sb[:].rearrange("p (kt d) -> p kt d", kt=8),
                          I["wqT"].rearrange("(kt p) d -> p kt d", p=128))
        enc_sb = resB.tile([128, 8 * 512], BF16)
        nc.sync.dma_start(enc_sb[:].rearrange("p (kt r) -> p kt r", kt=8),
                          I["encT"].rearrange("(kt p) r -> p kt r", p=128))
        vT_sb = resB.tile([128, 8], BF16)
        nc.sync.dma_start(vT_sb[:], I["vT"][:])
        bq_sb = resB.tile([128, 8], FP32)
        nc.sync.dma_start(bq_sb[:], I["bqT"][:])
        mask_sb = resB.tile([1, 512], FP32)
        nc.sync.dma_start(mask_sb[:], I["maskneg"][:])
        bhhn_sb = resB.tile([128, 8], FP32)
        nc.sync.dma_start(bhhn_sb[:], I["bhhnT"][:])
        alpha_sb = resB.tile([128, NSTEP], FP32)
        nc.sync.dma_start(alpha_sb[:], I["alpha"][:])
        h0_sb = resB.tile([128, 64], FP32)
        nc.sync.dma_start(h0_sb[:], I["h0T"][:])
        wtil = resB.tile([128, 32], BF16)  # block-diag softmax weights, 4 tiles
        nc.vector.memset(wtil[:], 0.0)
        whh_sb = resB.tile([128, 8 * GN], BF16)
        nc.sync.dma_start(whh_sb[:].rearrange("p (kt n) -> p kt n", kt=8),
                          I["whhT"].rearrange("(kt p) n -> p kt n", p=128))

        # ---------------- phase B: the GRU scan ----------------
        with tc.tile_pool(name="step", bufs=1) as stepp, \
                tc.tile_pool(name="ps_small", bufs=2, space="PSUM") as ps_s, \
                tc.tile_pool(name="ps_gate", bufs=2, space="PSUM") as ps_g, \
                tc.tile_pool(name="ps_tr", bufs=2, space="PSUM") as ps_t:
            for i in range(NSTEP):
                prev = h0_sb[:] if i == 0 else hist[:, (i - 1) * 64: i * 64]
                hbf = stepp.tile([128, 64], BF16, tag="hbf")
                nc.vector.tensor_copy(hbf[:], prev)

                # q = h @ Wq.T  -> [8, 1024] (rows = batch)
                q_sb = stepp.tile([8, 1024], FP32, tag="q")
                for nq in range(2):
                    pq = ps_s.tile([8, 512], FP32, tag="pq")
                    for k in range(8):
                        nc.tensor.matmul(
                            pq[:], hbf[:, k * 8:(k + 1) * 8],
                            wq_sb[:, k * H + nq * 512: k * H + (nq + 1) * 512],
                            start=(k == 0), stop=(k == 7))
                    if nq == 0:
                        nc.vector.tensor_copy(q_sb[:, nq * 512:(nq + 1) * 512], pq[:])
                    else:
                        nc.scalar.copy(q_sb[:, nq * 512:(nq + 1) * 512], pq[:])

                # qT [128, (dt, b)] + bq, cast bf16
                qtwt = ps_t.tile([128, 68], FP32, tag="qtwt")
                qt_ps = qtwt[:, 0:64]
                for dt in range(8):
                    nc.tensor.transpose(
                        qt_ps[:, dt * 8:(dt + 1) * 8],
                        q_sb[0:8, dt * 128:(dt + 1) * 128], ident[0:8, 0:8])
                qT = stepp.tile([128, 64], BF16, tag="qT")
                for dt in range(8):
                    nc.vector.tensor_scalar_add(
                        qT[:, dt * 8:(dt + 1) * 8],
                        qt_ps[:, dt * 8:(dt + 1) * 8], bq_sb[:, dt:dt + 1])

                # t = tanh(enc + q)   [128, (dt, b, s64)]
                t_in = stepp.tile([128, 4096], BF16, tag="tin")
                for dt in range(8):
                    nc.vector.tensor_add(
                        t_in[:, dt * 512:(dt + 1) * 512].rearrange(
                            "p (b s) -> p b s", b=8),
                        enc_sb[:, dt * 512:(dt + 1) * 512].rearrange(
                            "p (b s) -> p b s", b=8),
                        qT[:, dt * 8:(dt + 1) * 8].broadcast_to([128, 8, S64]))
                t_sb = stepp.tile([128, 4096], BF16, tag="tsb")
                nc.scalar.activation(t_sb[:], t_in[:], AF.Tanh)

                # e = v . t  -> [1, (b, s64)]
                pe = ps_s.tile([1, 512], FP32, tag="pq")
                for k in range(8):
                    nc.tensor.matmul(
                        pe[:], vT_sb[:, k:k + 1], t_sb[:, k * 512:(k + 1) * 512],
                        start=(k == 0), stop=(k == 7))

                # masked softmax over s (no max subtraction; |e| <= ~25)
                e_sb = stepp.tile([1, 512], FP32, tag="esb")
                nc.vector.tensor_add(e_sb[:], pe[:], mask_sb[:])
                x_sb = stepp.tile([1, 512], FP32, tag="xsb")
                nc.scalar.activation(x_sb[:], e_sb[:], AF.Exp)
                s_sb = stepp.tile([1, 8], FP32, tag="ssb")
                nc.vector.tensor_reduce(
                    s_sb[:], x_sb[:].rearrange("p (b s) -> p b s", b=8),
                    axis=AX.X, op=ALU.add)
                r_sb = stepp.tile([1, 8], FP32, tag="rsb")
                nc.vector.reciprocal(r_sb[:], s_sb[:])
                w_sb = stepp.tile([1, 512], FP32, tag="wsb")
                nc.vector.tensor_mul(
                    w_sb[:].rearrange("p (b s) -> p b s", b=8),
                    x_sb[:].rearrange("p (b s) -> p b s", b=8),
                    r_sb[:].broadcast_to([1, 8, S64]))

                # scatter w into block-diagonal wtil via PE transpose
                wt_ps = qtwt[:, 64:68]
                for c in range(4):
                    nc.tensor.transpose(
                        wt_ps[:, c:c + 1], w_sb[0:1, c * 128:(c + 1) * 128],
                        ident[0:1, 0:1])
                for c in range(4):
                    nc.vector.tensor_copy(
                        wtil[0:64, c * 8 + 2 * c: c * 8 + 2 * c + 1],
                        wt_ps[0:64, c:c + 1])
                    nc.vector.tensor_copy(
                        wtil[64:128, c * 8 + 2 * c + 1: c * 8 + 2 * c + 2],
                        wt_ps[64:128, c:c + 1])

                # gate pre-activations: rz combined (gh + gi), n separate
                rz_sb = stepp.tile([8, 2048], FP32, tag="rz")
                gin_sb = stepp.tile([8, 1024], FP32, tag="gin")
                ghn_sb = stepp.tile([8, 1024], FP32, tag="ghn")
                for ng in range(6):
                    if ng < 4:
                        pp = ps_g.tile([8, 512], FP32, tag="pre")
                        for k in range(8):
                            nc.tensor.matmul(
                                pp[:], hbf[:, k * 8:(k + 1) * 8],
                                whh_sb[:, k * GN + ng * 512: k * GN + (ng + 1) * 512],
                                start=(k == 0), stop=False)
                        for c in range(4):
                            nc.tensor.matmul(
                                pp[:], wtil[:, c * 8:(c + 1) * 8],
                                ctx2[:, c * GN + ng * 512: c * GN + (ng + 1) * 512],
                                start=False, stop=(c == 3))
                        if ng % 2 == 0:
                            nc.vector.tensor_copy(
                                rz_sb[:, ng * 512:(ng + 1) * 512], pp[:])
                        else:
                            nc.scalar.copy(rz_sb[:, ng * 512:(ng + 1) * 512], pp[:])
                    else:
                        pgh = ps_g.tile([8, 512], FP32, tag="pre")
                        for k in range(8):
                            nc.tensor.matmul(
                                pgh[:], hbf[:, k * 8:(k + 1) * 8],
                                whh_sb[:, k * GN + ng * 512: k * GN + (ng + 1) * 512],
                                start=(k == 0), stop=(k == 7))
                        nc.scalar.copy(
                            ghn_sb[:, (ng - 4) * 512:(ng - 3) * 512], pgh[:])
                        pgi = ps_g.tile([8, 512], FP32, tag="pre")
                        for c in range(4):
                            nc.tensor.matmul(
                                pgi[:], wtil[:, c * 8:(c + 1) * 8],
                                ctx2[:, c * GN + ng * 512: c * GN + (ng + 1) * 512],
                                start=(c == 0), stop=(c == 3))
                        nc.vector.tensor_copy(
                            gin_sb[:, (ng - 4) * 512:(ng - 3) * 512], pgi[:])

                # transpose pre-activations to [128, (nt, b)]
                tr_ps = ps_t.tile([128, 256], FP32, tag="tr")
                tr_r = tr_ps[:, 0:64]
                tr_z = tr_ps[:, 64:128]
                tr_gi = tr_ps[:, 128:192]
                tr_gh = tr_ps[:, 192:256]
                for nt in range(8):
                    nc.tensor.transpose(
                        tr_r[:, nt * 8:(nt + 1) * 8],
                        rz_sb[0:8, nt * 128:(nt + 1) * 128], ident[0:8, 0:8])
                    nc.tensor.transpose(
                        tr_z[:, nt * 8:(nt + 1) * 8],
                        rz_sb[0:8, 1024 + nt * 128: 1024 + (nt + 1) * 128],
                        ident[0:8, 0:8])
                    nc.tensor.transpose(
                        tr_gi[:, nt * 8:(nt + 1) * 8],
                        gin_sb[0:8, nt * 128:(nt + 1) * 128], ident[0:8, 0:8])
                    nc.tensor.transpose(
                        tr_gh[:, nt * 8:(nt + 1) * 8],
                        ghn_sb[0:8, nt * 128:(nt + 1) * 128], ident[0:8, 0:8])

                # gates
                rT = stepp.tile([128, 64], FP32, tag="rT")
                nc.scalar.activation(rT[:], tr_r, AF.Sigmoid)
                zT = stepp.tile([128, 64], FP32, tag="zT")
                nc.scalar.activation(zT[:], tr_z, AF.Sigmoid)
                tmp = stepp.tile([128, 64], FP32, tag="tmp")
                for nt in range(8):
                    nc.vector.tensor_scalar_add(
                        tmp[:, nt * 8:(nt + 1) * 8],
                        tr_gh[:, nt * 8:(nt + 1) * 8], bhhn_sb[:, nt:nt + 1])
                tmp2 = stepp.tile([128, 64], FP32, tag="tmp2")
                nc.vector.tensor_mul(tmp2[:], rT[:], tmp[:])
                pre_n = stepp.tile([128, 64], FP32, tag="pren")
                nc.vector.tensor_add(pre_n[:], tmp2[:], tr_gi)
                nT = stepp.tile([128, 64], FP32, tag="nT")
                nc.scalar.activation(nT[:], pre_n[:], AF.Tanh)
                t3 = stepp.tile([128, 64], FP32, tag="t3")
                nc.vector.tensor_sub(t3[:], prev, nT[:])
                t4 = stepp.tile([128, 64], FP32, tag="t4")
                nc.vector.tensor_mul(t4[:], zT[:], t3[:])
                t5 = stepp.tile([128, 64], FP32, tag="t5")
                nc.vector.tensor_add(t5[:], nT[:], t4[:])
                nc.vector.tensor_scalar_mul(
                    hist[:, i * 64:(i + 1) * 64], t5[:], alpha_sb[:, i:i + 1])

        ab_stack.close()

        # ---------------- phase C: projections, generator, log-softmax -------
        with tc.tile_pool(name="pc", bufs=1) as pc, \
                tc.tile_pool(name="pcw", bufs=2) as pcw, \
                tc.tile_pool(name="pcs", bufs=4) as pcs, \
                tc.tile_pool(name="pcps", bufs=4, space="PSUM") as pcps:
            wout_sb = pc.tile([128, 8 * E], FP32)
            nc.sync.dma_start(
                wout_sb[:].rearrange("p (kt e) -> p kt e", kt=8),
                I["woutT"].rearrange("(kt p) e -> p kt e", p=128))
            wemb_sb = pc.tile([128, 4 * E], FP32)
            nc.sync.dma_start(
                wemb_sb[:].rearrange("p (kt e) -> p kt e", kt=4),
                I["wembT"].rearrange("(kt p) e -> p kt e", p=128))
            embsel_sb = pc.tile([128, 4 * ROWS], FP32)
            nc.sync.dma_start(
                embsel_sb[:].rearrange("p (kt r) -> p kt r", kt=4),
                I["embTsel"].rearrange("(kt p) r -> p kt r", p=128))
            o0_sb = pc.tile([128, 4 * ROWS], FP32)
            nc.sync.dma_start(
                o0_sb[:].rearrange("p (kt r) -> p kt r", kt=4),
                I["o0T"].rearrange("(kt p) r -> p kt r", p=128))

            fullT = pc.tile([128, 4 * ROWS], BF16)
            hist_v = hist[:].rearrange("p (st jt b) -> p st jt b", st=NSTEP, jt=8)
            for et in range(4):
                pf = pcps.tile([128, 256], FP32, tag="pf")
                for jt in range(8):
                    nc.tensor.matmul(
                        pf[:], wout_sb[:, jt * E + et * 128: jt * E + (et) * 128 + 128],
                        hist_v[:, :, jt: jt + 1, :], start=(jt == 0), stop=False)
                for kt in range(4):
                    nc.tensor.matmul(
                        pf[:], wemb_sb[:, kt * E + et * 128: kt * E + et * 128 + 128],
                        embsel_sb[:, kt * ROWS:(kt + 1) * ROWS],
                        start=False, stop=(kt == 3))
                nc.vector.tensor_add(
                    fullT[:, et * ROWS:(et + 1) * ROWS], pf[:],
                    o0_sb[:, et * ROWS:(et + 1) * ROWS])

            sums = pc.tile([128, 2 * NVC], FP32)
            lgts = pc.tile([128, 2 * V], BF16)
            neglse = pc.tile([128, 2], FP32)

            for vc in range(NVC):
                w = _vchunk(vc)
                wgs = []
                for et in range(4):
                    wg = pcw.tile([128, 512], BF16, tag=f"wg{et}", name=f"wg{vc}_{et}")
                    nc.sync.dma_start(
                        wg[:, :w],
                        I["wgenT"][et * 128:(et + 1) * 128, vc * 512: vc * 512 + w])
                    wgs.append(wg)
                for mc in range(2):
                    pl = pcps.tile([128, 512], FP32, tag="pl")
                    for et in range(4):
                        nc.tensor.matmul(
                            pl[:, :w],
                            fullT[:, et * ROWS + mc * 128: et * ROWS + mc * 128 + 128],
                            wgs[et][:, :w], start=(et == 0), stop=(et == 3))
                    scr = pcs.tile([128, 512], BF16, tag="scr")
                    nc.scalar.activation(
                        scr[:, :w], pl[:, :w], AF.Exp,
                        accum_out=sums[:, mc * NVC + vc: mc * NVC + vc + 1])
                    nc.vector.tensor_copy(
                        lgts[:, mc * V + vc * 512: mc * V + vc * 512 + w], pl[:, :w])

            for mc in range(2):
                ssum = pcs.tile([128, 1], FP32, tag="ssum")
                nc.vector.tensor_reduce(
                    ssum[:], sums[:, mc * NVC:(mc + 1) * NVC], axis=AX.X, op=ALU.add)
                lse_t = pcs.tile([128, 1], FP32, tag="lse")
                nc.scalar.activation(lse_t[:], ssum[:], AF.Ln)
                nc.vector.tensor_scalar_mul(neglse[:, mc:mc + 1], lse_t[:], -1.0)

            for mc in range(2):
                for vc in range(NVC):
                    w = _vchunk(vc)
                    ot = pcs.tile([128, 512], FP32, tag="ot")
                    nc.vector.tensor_scalar_add(
                        ot[:, :w],
                        lgts[:, mc * V + vc * 512: mc * V + vc * 512 + w],
                        neglse[:, mc:mc + 1])
                    nc.sync.dma_start(
                        out[mc * 128:(mc + 1) * 128, vc * 512: vc * 512 + w],
                        ot[:, :w])


INPUT_SPECS = [
    ("ctxT", (C, 512), BF16),
    ("wihT", (C, GN), BF16),
    ("biasf", (128, GN), FP32),
    ("whhT", (H, GN), BF16),
    ("wqT", (H, H), BF16),
    ("encT", (H, 512), BF16),
    ("vT", (128, 8), BF16),
    ("bqT", (128, 8), FP32),
    ("maskneg", (1, 512), FP32),
    ("h0T", (128, 64), FP32),
    ("bhhnT", (128, 8), FP32),
    ("alpha", (128, NSTEP), FP32),
    ("woutT", (H, E), FP32),
    ("wembT", (E, E), FP32),
    ("embTsel", (E, ROWS), FP32),
    ("o0T", (E, ROWS), FP32),
    ("wgenT", (E, V), BF16),
]


def build_program(n_cores=N_CORES):
    nc = bacc.Bacc(
        "TRN2", target_bir_lowering=False, debug=False, num_devices=n_cores)
    ins = {}
    for name, shape, d in INPUT_SPECS:
        ins[name] = nc.dram_tensor(name, list(shape), d, kind="ExternalInput").ap()
    out = nc.dram_tensor("out", [ROWS, V], FP32, kind="ExternalOutput").ap()
    with tile.TileContext(nc) as tc:
        decoder_kernel(tc, out, ins)
    nc.compile()
    return nc


def _t_list(stream):
    """Output time-slot for each of the 32 row-groups ti of a core."""
    if stream == 0:
        return [2 * ti + 1 for ti in range(NSTEP)]
    return [2 * ti + 2 for ti in range(NSTEP - 1)] + [0]


def _to_tiles_T(mat):
    """[K, N] -> [128, (kt, N)] tile-major layout for DMA-free slicing checks."""
    K, N = mat.shape
    return np.ascontiguousarray(mat.reshape(K // 128, 128, N).transpose(1, 0, 2)
                                .reshape(128, -1))


def prep_core_inputs(core, inp):
    """Build the per-core input map (all numpy, host-side shard/transpose)."""
    f32 = np.float32
    bf16 = ml_dtypes.bfloat16
    st, g = core // 4, core % 4
    bg = slice(g * BL, (g + 1) * BL)
    sfx = "r" if st == 0 else "l"

    ctx = np.asarray(inp["ctx"], f32)[bg]            # [8, S, C]
    enc = np.asarray(inp["enc_cache"], f32)[bg]      # [8, S, H]
    maskf = np.asarray(inp["ctx_mask"], f32)[bg]     # [8, S]
    h0 = np.asarray(inp["hiddens"], f32)[bg, 0]      # [8, H]
    emb = np.asarray(inp["embeddings"], f32)[bg]     # [8, T, E]
    o0 = np.asarray(inp["outputs0"], f32)[bg]        # [8, E]

    Wq = np.asarray(inp["Wq"], f32)
    bq = np.asarray(inp["bq"], f32)
    v_att = np.asarray(inp["v_att"], f32)
    Wih = np.asarray(inp[f"Wih_{sfx}"], f32)
    Whh = np.asarray(inp[f"Whh_{sfx}"], f32)
    bih = np.asarray(inp[f"bih_{sfx}"], f32)
    bhh = np.asarray(inp[f"bhh_{sfx}"], f32)
    Wout = np.asarray(inp["Wout"], f32)
    bout = np.asarray(inp["bout"], f32)
    Wemb = np.asarray(inp["Wemb"], f32)
    bemb = np.asarray(inp["bemb"], f32)
    Wgen = np.asarray(inp["Wgen"], f32)

    ctxT = np.zeros((C, 512), f32)
    encT = np.zeros((H, 512), f32)
    maskneg = np.full((1, 512), -1e9, f32)
    for b in range(BL):
        ctxT[:, b * 64: b * 64 + S] = ctx[b].T
        encT[:, b * 64: b * 64 + S] = enc[b].T
        maskneg[0, b * 64: b * 64 + S] = np.where(maskf[b] > 0.5, -1e9, 0.0)

    bias_fold = bih + np.concatenate([bhh[:2 * H], np.zeros(H, f32)])
    biasf = np.broadcast_to(bias_fold[None, :], (128, GN)).copy()

    h0T = np.ascontiguousarray(
        h0.T.reshape(8, 128, BL).transpose(1, 0, 2).reshape(128, 64))
    bqT = np.ascontiguousarray(bq.reshape(8, 128).T)
    vT = np.ascontiguousarray(v_att.reshape(8, 128).T)
    bhhnT = np.ascontiguousarray(bhh[2 * H:].reshape(8, 128).T)

    alpha = np.ones((128, NSTEP), f32)
    if st == 1:
        alpha[:, NSTEP - 1] = 0.0

    ts = _t_list(st)
    embTsel = np.zeros((E, ROWS), f32)
    o0T = np.zeros((E, ROWS), f32)
    bb = bout + bemb
    for ti, t in enumerate(ts):
        for b in range(BL):
            r = ti * BL + b
            if t == 0:
                o0T[:, r] = o0[b]
            else:
                o0T[:, r] = bb
                embTsel[:, r] = emb[b, t - 1]

    return {
        "ctxT": ctxT.astype(bf16),
        "wihT": np.ascontiguousarray(Wih.T).astype(bf16),
        "biasf": biasf,
        "whhT": np.ascontiguousarray(Whh.T).astype(bf16),
        "wqT": np.ascontiguousarray(Wq.T).astype(bf16),
        "encT": encT.astype(bf16),
        "vT": vT.astype(bf16),
        "bqT": bqT,
        "maskneg": maskneg,
        "h0T": h0T,
        "bhhnT": bhhnT,
        "alpha": alpha,
        "woutT": np.ascontiguousarray(Wout.T),
        "wembT": np.ascontiguousarray(Wemb.T),
        "embTsel": embTsel,
        "o0T": o0T,
        "wgenT": np.ascontiguousarray(Wgen.T).astype(bf16),
    }


def assemble_output(core_outs):
    """core_outs: list of 8 arrays [ROWS, V] -> full [B, T, V]."""
    res = np.empty((B, T, V), np.float32)
    for core in range(N_CORES):
        st, g = core // 4, core % 4
        ts = _t_list(st)
        o = core_outs[core].reshape(NSTEP, BL, V)
        for ti, t in enumerate(ts):
            res[g * BL:(g + 1) * BL, t, :] = o[ti]
    return res


_PROG = None


def _get_prog():
    global _PROG
    if _PROG is None:
        _PROG = build_program(N_CORES)
    return _PROG


def kernel(**inputs) -> np.ndarray:
    nc = _get_prog()
    in_maps = [prep_core_inputs(c, inputs) for c in range(N_CORES)]
    res = bass_utils.run_bass_kernel_spmd(nc, in_maps, core_ids=list(range(N_CORES)))
    return assemble_output([res.results[c]["out"] for c in range(N_CORES)])



# revision 21
# speedup vs baseline: 7857.3401x; 7857.3401x over previous
"""Trainium2 Bass kernel for nn_Decoder (Bahdanau-attention GRU decoder).

Sharding: 8 cores = 2 streams (right/left interleaved GRU scans) x 4 batch
groups (8 batches each).  Zero collectives; each core computes its stream's
hidden states for its batches and the full generator/log-softmax for its
output rows.

Per-core phases:
  A: ctx2 = ctx @ Wih.T + bias_fold     (attention context folded into gate
     space, so the per-step attention-weighted sum becomes a small einsum)
  B: 32 GRU steps (left stream's last step is a dummy, h zeroed via alpha)
  C: out-projection + embedding projection + generator matmul + log-softmax
"""

import os
import sys

import numpy as np

for _p in ("/opt/trn_rl_repo", "/root/.axon_site/_ro/trn_rl_repo"):
    if os.path.isdir(_p) and _p not in sys.path:
        sys.path.append(_p)

import ml_dtypes  # noqa: E402

import concourse.bass as bass  # noqa: E402
import concourse.tile as tile  # noqa: E402
from concourse import bacc, masks, mybir  # noqa: E402
from concourse import bass_utils  # noqa: E402

FP32 = mybir.dt.float32
BF16 = mybir.dt.bfloat16
AF = mybir.ActivationFunctionType
ALU = mybir.AluOpType
AX = mybir.AxisListType

B, S, T = 32, 48, 64
H, C, E, V = 1024, 2048, 512, 32000
S64 = 64          # padded source length
BL = 8            # batches per core
NSTEP = 32        # GRU steps per core (left runs one dummy step)
ROWS = NSTEP * BL  # 256 output rows per core
GN = 3 * H        # 3072 gate rows
N_CORES = 8
NVC = 63          # 62 full 512-chunks + one 256 chunk of V


def _vchunk(vc):
    return 512 if vc < 62 else 256


def decoder_kernel(tc: tile.TileContext, out: bass.AP, I: dict):
    nc = tc.nc

    from contextlib import ExitStack

    with ExitStack() as ctx:
        const = ctx.enter_context(tc.tile_pool(name="const", bufs=1))
        ident = const.tile([128, 128], FP32)
        masks.make_identity(nc, ident[:])

        pers = ctx.enter_context(tc.tile_pool(name="pers", bufs=1))
        hist = pers.tile([128, NSTEP * 64], FP32)  # slot i: [:, i*64+(jt*8+b)]

        ab_stack = ExitStack()
        ctx2pool = ab_stack.enter_context(tc.tile_pool(name="ctx2p", bufs=1))
        ctx2 = ctx2pool.tile([128, 4 * GN], BF16)  # m-tile-major, (m, n)

        # ---------------- phase A: ctx2 = ctxT.T @ wihT + bias ----------------
        with tc.tile_pool(name="pa", bufs=1) as pa, \
                tc.tile_pool(name="paw", bufs=4) as paw, \
                tc.tile_pool(name="paps", bufs=2, space="PSUM") as paps:
            ctxT_sb = pa.tile([128, 16 * 512], BF16)
            nc.sync.dma_start(
                ctxT_sb[:].rearrange("p (kt r) -> p kt r", kt=16),
                I["ctxT"].rearrange("(kt p) r -> p kt r", p=128))
            bias_bc = pa.tile([128, GN], FP32)
            nc.sync.dma_start(bias_bc[:], I["biasf"][:])

            for ng in range(6):
                psums = [paps.tile([128, 512], FP32, tag=f"pa{m}", name=f"pa{ng}_{m}")
                         for m in range(4)]
                for k in range(16):
                    wt = paw.tile([128, 512], BF16, tag="wihtile")
                    nc.sync.dma_start(
                        wt[:], I["wihT"][k * 128:(k + 1) * 128, ng * 512:(ng + 1) * 512])
                    for m in range(4):
                        nc.tensor.matmul(
                            psums[m][:],
                            ctxT_sb[:, k * 512 + m * 128: k * 512 + (m + 1) * 128],
                            wt[:], start=(k == 0), stop=(k == 15))
                for m in range(4):
                    nc.vector.tensor_add(
                        ctx2[:, m * GN + ng * 512: m * GN + (ng + 1) * 512],
                        psums[m][:], bias_bc[:, ng * 512:(ng + 1) * 512])

        # ---------------- phase B residents ----------------
        resB = ab_stack.enter_context(tc.tile_pool(name="resB", bufs=1))
        wq_sb = resB.tile([128, 8 * H], BF16)
        nc.sync.dma_start(wq_sb[:].rearrange("p (kt d) -> p kt d", kt=8),
                          I["wqT"].rearrange("(kt p) d -> p kt d", p=128))
        enc_sb = resB.tile([128, 8 * 512], BF16)
        nc.sync.dma_start(enc_sb[:].rearrange("p (kt r) -> p kt r", kt=8),
                          I["encT"].rearrange("(kt p) r -> p kt r", p=128))
        vT_sb = resB.tile([128, 8], BF16)
        nc.sync.dma_start(vT_sb[:], I["vT"][:])
        bq_sb = resB.tile([128, 8], FP32)
        nc.sync.dma_start(bq_sb[:], I["bqT"][:])
        mask_sb = resB.tile([1, 512], FP32)
        nc.sync.dma_start(mask_sb[:], I["maskneg"][:])
        bhhn_sb = resB.tile([128, 8], FP32)
        nc.sync.dma_start(bhhn_sb[:], I["bhhnT"][:])
        alpha_sb = resB.tile([128, NSTEP], FP32)
        nc.sync.dma_start(alpha_sb[:], I["alpha"][:])
        h0_sb = resB.tile([128, 64], FP32)
        nc.sync.dma_start(h0_sb[:], I["h0T"][:])
        wtil = resB.tile([128, 32], BF16)  # block-diag softmax weights, 4 tiles
        nc.vector.memset(wtil[:], 0.0)
        whh_sb = resB.tile([128, 8 * GN], BF16)
        nc.sync.dma_start(whh_sb[:].rearrange("p (kt n) -> p kt n", kt=8),
                          I["whhT"].rearrange("(kt p) n -> p kt n", p=128))

        # ---------------- phase B: the GRU scan ----------------
        with tc.tile_pool(name="step", bufs=1) as stepp, \
                tc.tile_pool(name="ps_small", bufs=2, space="PSUM") as ps_s, \
                tc.tile_pool(name="ps_gate", bufs=2, space="PSUM") as ps_g, \
                tc.tile_pool(name="ps_tr", bufs=2, space="PSUM") as ps_t:
            for i in range(NSTEP):
                prev = h0_sb[:] if i == 0 else hist[:, (i - 1) * 64: i * 64]
                hbf = stepp.tile([128, 64], BF16, tag="hbf")
                nc.vector.tensor_copy(hbf[:], prev)

                # q = h @ Wq.T  -> [8, 1024] (rows = batch)
                q_sb = stepp.tile([8, 1024], FP32, tag="q")
                for nq in range(2):
                    pq = ps_s.tile([8, 512], FP32, tag="pq")
                    for k in range(8):
                        nc.tensor.matmul(
                            pq[:], hbf[:, k * 8:(k + 1) * 8],
                            wq_sb[:, k * H + nq * 512: k * H + (nq + 1) * 512],
                            start=(k == 0), stop=(k == 7))
                    if nq == 0:
                        nc.vector.tensor_copy(q_sb[:, nq * 512:(nq + 1) * 512], pq[:])
                    else:
                        nc.scalar.copy(q_sb[:, nq * 512:(nq + 1) * 512], pq[:])

                # qT [128, (dt, b)] + bq, cast bf16
                qtwt = ps_t.tile([128, 68], FP32, tag="qtwt")
                qt_ps = qtwt[:, 0:64]
                for dt in range(8):
                    nc.tensor.transpose(
                        qt_ps[:, dt * 8:(dt + 1) * 8],
                        q_sb[0:8, dt * 128:(dt + 1) * 128], ident[0:8, 0:8])
                qT = stepp.tile([128, 64], BF16, tag="qT")
                for dt in range(8):
                    nc.vector.tensor_scalar_add(
                        qT[:, dt * 8:(dt + 1) * 8],
                        qt_ps[:, dt * 8:(dt + 1) * 8], bq_sb[:, dt:dt + 1])

                # t = tanh(enc + q)   [128, (dt, b, s64)]
                t_in = stepp.tile([128, 4096], BF16, tag="tin")
                for dt in range(8):
                    nc.vector.tensor_add(
                        t_in[:, dt * 512:(dt + 1) * 512].rearrange(
                            "p (b s) -> p b s", b=8),
                        enc_sb[:, dt * 512:(dt + 1) * 512].rearrange(
                            "p (b s) -> p b s", b=8),
                        qT[:, dt * 8:(dt + 1) * 8].broadcast_to([128, 8, S64]))
                t_sb = stepp.tile([128, 4096], BF16, tag="tsb")
                nc.scalar.activation(t_sb[:], t_in[:], AF.Tanh)

                # e = v . t  -> [1, (b, s64)]
                pe = ps_s.tile([1, 512], FP32, tag="pq")
                for k in range(8):
                    nc.tensor.matmul(
                        pe[:], vT_sb[:, k:k + 1], t_sb[:, k * 512:(k + 1) * 512],
                        start=(k == 0), stop=(k == 7))

                # masked softmax over s (no max subtraction; |e| <= ~25)
                e_sb = stepp.tile([1, 512], FP32, tag="esb")
                nc.vector.tensor_add(e_sb[:], pe[:], mask_sb[:])
                x_sb = stepp.tile([1, 512], FP32, tag="xsb")
                nc.scalar.activation(x_sb[:], e_sb[:], AF.Exp)
                s_sb = stepp.tile([1, 8], FP32, tag="ssb")
                nc.vector.tensor_reduce(
                    s_sb[:], x_sb[:].rearrange("p (b s) -> p b s", b=8),
                    axis=AX.X, op=ALU.add)
                r_sb = stepp.tile([1, 8], FP32, tag="rsb")
                nc.vector.reciprocal(r_sb[:], s_sb[:])
                w_sb = stepp.tile([1, 512], FP32, tag="wsb")
                nc.vector.tensor_mul(
                    w_sb[:].rearrange("p (b s) -> p b s", b=8),
                    x_sb[:].rearrange("p (b s) -> p b s", b=8),
                    r_sb[:].broadcast_to([1, 8, S64]))

                # scatter w into block-diagonal wtil via PE transpose
                wt_ps = qtwt[:, 64:68]
                for c in range(4):
                    nc.tensor.transpose(
                        wt_ps[:, c:c + 1], w_sb[0:1, c * 128:(c + 1) * 128],
                        ident[0:1, 0:1])
                for c in range(4):
                    nc.vector.tensor_copy(
                        wtil[0:64, c * 8 + 2 * c: c * 8 + 2 * c + 1],
                        wt_ps[0:64, c:c + 1])
                    nc.vector.tensor_copy(
                        wtil[64:128, c * 8 + 2 * c + 1: c * 8 + 2 * c + 2],
                        wt_ps[64:128, c:c + 1])

                # gate pre-activations: rz combined (gh + gi), n separate
                rz_sb = stepp.tile([8, 2048], FP32, tag="rz")
                gin_sb = stepp.tile([8, 1024], FP32, tag="gin")
                ghn_sb = stepp.tile([8, 1024], FP32, tag="ghn")
                for ng in range(6):
                    if ng < 4:
                        pp = ps_g.tile([8, 512], FP32, tag="pre")
                        for k in range(8):
                            nc.tensor.matmul(
                                pp[:], hbf[:, k * 8:(k + 1) * 8],
                                whh_sb[:, k * GN + ng * 512: k * GN + (ng + 1) * 512],
                                start=(k == 0), stop=False)
                        for c in range(4):
                            nc.tensor.matmul(
                                pp[:], wtil[:, c * 8:(c + 1) * 8],
                                ctx2[:, c * GN + ng * 512: c * GN + (ng + 1) * 512],
                                start=False, stop=(c == 3))
                        if ng % 2 == 0:
                            nc.vector.tensor_copy(
                                rz_sb[:, ng * 512:(ng + 1) * 512], pp[:])
                        else:
                            nc.scalar.copy(rz_sb[:, ng * 512:(ng + 1) * 512], pp[:])
                    else:
                        pgh = ps_g.tile([8, 512], FP32, tag="pre")
                        for k in range(8):
                            nc.tensor.matmul(
                                pgh[:], hbf[:, k * 8:(k + 1) * 8],
                                whh_sb[:, k * GN + ng * 512: k * GN + (ng + 1) * 512],
                                start=(k == 0), stop=(k == 7))
                        nc.scalar.copy(
                            ghn_sb[:, (ng - 4) * 512:(ng - 3) * 512], pgh[:])
                        pgi = ps_g.tile([8, 512], FP32, tag="pre")
                        for c in range(4):
                            nc.tensor.matmul(
                                pgi[:], wtil[:, c * 8:(c + 1) * 8],
                                ctx2[:, c * GN + ng * 512: c * GN + (ng + 1) * 512],
                                start=(c == 0), stop=(c == 3))
                        nc.vector.tensor_copy(
                            gin_sb[:, (ng - 4) * 512:(ng - 3) * 512], pgi[:])

                # transpose pre-activations to [128, (nt, b)]
                tr_ps = ps_t.tile([128, 256], FP32, tag="tr")
                tr_r = tr_ps[:, 0:64]
                tr_z = tr_ps[:, 64:128]
                tr_gi = tr_ps[:, 128:192]
                tr_gh = tr_ps[:, 192:256]
                for nt in range(8):
                    nc.tensor.transpose(
                        tr_r[:, nt * 8:(nt + 1) * 8],
                        rz_sb[0:8, nt * 128:(nt + 1) * 128], ident[0:8, 0:8])
                    nc.tensor.transpose(
                        tr_z[:, nt * 8:(nt + 1) * 8],
                        rz_sb[0:8, 1024 + nt * 128: 1024 + (nt + 1) * 128],
                        ident[0:8, 0:8])
                    nc.tensor.transpose(
                        tr_gi[:, nt * 8:(nt + 1) * 8],
                        gin_sb[0:8, nt * 128:(nt + 1) * 128], ident[0:8, 0:8])
                    nc.tensor.transpose(
                        tr_gh[:, nt * 8:(nt + 1) * 8],
                        ghn_sb[0:8, nt * 128:(nt + 1) * 128], ident[0:8, 0:8])

                # gates
                rT = stepp.tile([128, 64], FP32, tag="rT")
                nc.scalar.activation(rT[:], tr_r, AF.Sigmoid)
                zT = stepp.tile([128, 64], FP32, tag="zT")
                nc.scalar.activation(zT[:], tr_z, AF.Sigmoid)
                tmp = stepp.tile([128, 64], FP32, tag="tmp")
                for nt in range(8):
                    nc.vector.tensor_scalar_add(
                        tmp[:, nt * 8:(nt + 1) * 8],
                        tr_gh[:, nt * 8:(nt + 1) * 8], bhhn_sb[:, nt:nt + 1])
                tmp2 = stepp.tile([128, 64], FP32, tag="tmp2")
                nc.vector.tensor_mul(tmp2[:], rT[:], tmp[:])
                pre_n = stepp.tile([128, 64], FP32, tag="pren")
                nc.vector.tensor_add(pre_n[:], tmp2[:], tr_gi)
                nT = stepp.tile([128, 64], FP32, tag="nT")
                nc.scalar.activation(nT[:], pre_n[:], AF.Tanh)
                t3 = stepp.tile([128, 64], FP32, tag="t3")
                nc.vector.tensor_sub(t3[:], prev, nT[:])
                t4 = stepp.tile([128, 64], FP32, tag="t4")
                nc.vector.tensor_mul(t4[:], zT[:], t3[:])
                t5 = stepp.tile([128, 64], FP32, tag="t5")
                nc.vector.tensor_add(t5[:], nT[:], t4[:])
                nc.vector.tensor_scalar_mul(
                    hist[:, i * 64:(i + 1) * 64], t5[:], alpha_sb[:, i:i + 1])

        ab_stack.close()

        # ---------------- phase C: projections, generator, log-softmax -------
        with tc.tile_pool(name="pc", bufs=1) as pc, \
                tc.tile_pool(name="pcw", bufs=2) as pcw, \
                tc.tile_pool(name="pcs", bufs=4) as pcs, \
                tc.tile_pool(name="pcps", bufs=4, space="PSUM") as pcps:
            wout_sb = pc.tile([128, 8 * E], FP32)
            nc.sync.dma_start(
                wout_sb[:].rearrange("p (kt e) -> p kt e", kt=8),
                I["woutT"].rearrange("(kt p) e -> p kt e", p=128))
            wemb_sb = pc.tile([128, 4 * E], FP32)
            nc.sync.dma_start(
                wemb_sb[:].rearrange("p (kt e) -> p kt e", kt=4),
                I["wembT"].rearrange("(kt p) e -> p kt e", p=128))
            embsel_sb = pc.tile([128, 4 * ROWS], FP32)
            nc.sync.dma_start(
                embsel_sb[:].rearrange("p (kt r) -> p kt r", kt=4),
                I["embTsel"].rearrange("(kt p) r -> p kt r", p=128))
            o0_sb = pc.tile([128, 4 * ROWS], FP32)
            nc.sync.dma_start(
                o0_sb[:].rearrange("p (kt r) -> p kt r", kt=4),
                I["o0T"].rearrange("(kt p) r -> p kt r", p=128))

            fullT = pc.tile([128, 4 * ROWS], BF16)
            hist_v = hist[:].rearrange("p (st jt b) -> p st jt b", st=NSTEP, jt=8)
            for et in range(4):
                pf = pcps.tile([128, 256], FP32, tag="pf")
                for jt in range(8):
                    nc.tensor.matmul(
                        pf[:], wout_sb[:, jt * E + et * 128: jt * E + (et) * 128 + 128],
                        hist_v[:, :, jt: jt + 1, :], start=(jt == 0), stop=False)
                for kt in range(4):
                    nc.tensor.matmul(
                        pf[:], wemb_sb[:, kt * E + et * 128: kt * E + et * 128 + 128],
                        embsel_sb[:, kt * ROWS:(kt + 1) * ROWS],
                        start=False, stop=(kt == 3))
                nc.vector.tensor_add(
                    fullT[:, et * ROWS:(et + 1) * ROWS], pf[:],
                    o0_sb[:, et * ROWS:(et + 1) * ROWS])

            sums = pc.tile([128, 2 * NVC], FP32)
            lgts = pc.tile([128, 2 * V], BF16)
            neglse = pc.tile([128, 2], FP32)

            for vc in range(NVC):
                w = _vchunk(vc)
                wgs = []
                for et in range(4):
                    wg = pcw.tile([128, 512], BF16, tag=f"wg{et}", name=f"wg{vc}_{et}")
                    nc.sync.dma_start(
                        wg[:, :w],
                        I["wgenT"][et * 128:(et + 1) * 128, vc * 512: vc * 512 + w])
                    wgs.append(wg)
                for mc in range(2):
                    pl = pcps.tile([128, 512], FP32, tag="pl")
                    for et in range(4):
                        nc.tensor.matmul(
                            pl[:, :w],
                            fullT[:, et * ROWS + mc * 128: et * ROWS + mc * 128 + 128],
                            wgs[et][:, :w], start=(et == 0), stop=(et == 3))
                    scr = pcs.tile([128, 512], BF16, tag="scr")
                    nc.scalar.activation(
                        scr[:, :w], pl[:, :w], AF.Exp,
                        accum_out=sums[:, mc * NVC + vc: mc * NVC + vc + 1])
                    nc.vector.tensor_copy(
                        lgts[:, mc * V + vc * 512: mc * V + vc * 512 + w], pl[:, :w])

            for mc in range(2):
                ssum = pcs.tile([128, 1], FP32, tag="ssum")
                nc.vector.tensor_reduce(
                    ssum[:], sums[:, mc * NVC:(mc + 1) * NVC], axis=AX.X, op=ALU.add)
                lse_t = pcs.tile([128, 1], FP32, tag="lse")
                nc.scalar.activation(lse_t[:], ssum[:], AF.Ln)
                nc.vector.tensor_scalar_mul(neglse[:, mc:mc + 1], lse_t[:], -1.0)

            for mc in range(2):
                for vc in range(NVC):
                    w = _vchunk(vc)
                    ot = pcs.tile([128, 512], FP32, tag="ot")
                    nc.vector.tensor_scalar_add(
                        ot[:, :w],
                        lgts[:, mc * V + vc * 512: mc * V + vc * 512 + w],
                        neglse[:, mc:mc + 1])
                    nc.sync.dma_start(
                        out[mc * 128:(mc + 1) * 128, vc * 512: vc * 512 + w],
                        ot[:, :w])


INPUT_SPECS = [
    ("ctxT", (C, 512), BF16),
    ("wihT", (C, GN), BF16),
    ("biasf", (128, GN), FP32),
    ("whhT", (H, GN), BF16),
    ("wqT", (H, H), BF16),
    ("encT", (H, 512), BF16),
    ("vT", (128, 8), BF16),
    ("bqT", (128, 8), FP32),
    ("maskneg", (1, 512), FP32),
    ("h0T", (128, 64), FP32),
    ("bhhnT", (128, 8), FP32),
    ("alpha", (128, NSTEP), FP32),
    ("woutT", (H, E), FP32),
    ("wembT", (E, E), FP32),
    ("embTsel", (E, ROWS), FP32),
    ("o0T", (E, ROWS), FP32),
    ("wgenT", (E, V), BF16),
]


def build_program(n_cores=N_CORES):
    nc = bacc.Bacc(
        "TRN2", target_bir_lowering=False, debug=False, num_devices=n_cores)
    ins = {}
    for name, shape, d in INPUT_SPECS:
        ins[name] = nc.dram_tensor(name, list(shape), d, kind="ExternalInput").ap()
    out = nc.dram_tensor("out", [ROWS, V], FP32, kind="ExternalOutput").ap()
    with tile.TileContext(nc) as tc:
        decoder_kernel(tc, out, ins)
    nc.compile()
    return nc


def _t_list(stream):
    """Output time-slot for each of the 32 row-groups ti of a core."""
    if stream == 0:
        return [2 * ti + 1 for ti in range(NSTEP)]
    return [2 * ti + 2 for ti in range(NSTEP - 1)] + [0]


def _to_tiles_T(mat):
    """[K, N] -> [128, (kt, N)] tile-major layout for DMA-free slicing checks."""
    K, N = mat.shape
    return np.ascontiguousarray(mat.reshape(K // 128, 128, N).transpose(1, 0, 2)
                                .reshape(128, -1))


def prep_core_inputs(core, inp):
    """Build the per-core input map (all numpy, host-side shard/transpose)."""
    f32 = np.float32
    bf16 = ml_dtypes.bfloat16
    st, g = core // 4, core % 4
    bg = slice(g * BL, (g + 1) * BL)
    sfx = "r" if st == 0 else "l"

    ctx = np.asarray(inp["ctx"], f32)[bg]            # [8, S, C]
    enc = np.asarray(inp["enc_cache"], f32)[bg]      # [8, S, H]
    maskf = np.asarray(inp["ctx_mask"], f32)[bg]     # [8, S]
    h0 = np.asarray(inp["hiddens"], f32)[bg, 0]      # [8, H]
    emb = np.asarray(inp["embeddings"], f32)[bg]     # [8, T, E]
    o0 = np.asarray(inp["outputs0"], f32)[bg]        # [8, E]

    Wq = np.asarray(inp["Wq"], f32)
    bq = np.asarray(inp["bq"], f32)
    v_att = np.asarray(inp["v_att"], f32)
    Wih = np.asarray(inp[f"Wih_{sfx}"], f32)
    Whh = np.asarray(inp[f"Whh_{sfx}"], f32)
    bih = np.asarray(inp[f"bih_{sfx}"], f32)
    bhh = np.asarray(inp[f"bhh_{sfx}"], f32)
    Wout = np.asarray(inp["Wout"], f32)
    bout = np.asarray(inp["bout"], f32)
    Wemb = np.asarray(inp["Wemb"], f32)
    bemb = np.asarray(inp["bemb"], f32)
    Wgen = np.asarray(inp["Wgen"], f32)

    ctxT = np.zeros((C, 512), f32)
    encT = np.zeros((H, 512), f32)
    maskneg = np.full((1, 512), -1e9, f32)
    for b in range(BL):
        ctxT[:, b * 64: b * 64 + S] = ctx[b].T
        encT[:, b * 64: b * 64 + S] = enc[b].T
        maskneg[0, b * 64: b * 64 + S] = np.where(maskf[b] > 0.5, -1e9, 0.0)

    bias_fold = bih + np.concatenate([bhh[:2 * H], np.zeros(H, f32)])
    biasf = np.broadcast_to(bias_fold[None, :], (128, GN)).copy()

    h0T = np.ascontiguousarray(
        h0.T.reshape(8, 128, BL).transpose(1, 0, 2).reshape(128, 64))
    bqT = np.ascontiguousarray(bq.reshape(8, 128).T)
    vT = np.ascontiguousarray(v_att.reshape(8, 128).T)
    bhhnT = np.ascontiguousarray(bhh[2 * H:].reshape(8, 128).T)

    alpha = np.ones((128, NSTEP), f32)
    if st == 1:
        alpha[:, NSTEP - 1] = 0.0

    ts = _t_list(st)
    embTsel = np.zeros((E, ROWS), f32)
    o0T = np.zeros((E, ROWS), f32)
    bb = bout + bemb
    for ti, t in enumerate(ts):
        for b in range(BL):
            r = ti * BL + b
            if t == 0:
                o0T[:, r] = o0[b]
            else:
                o0T[:, r] = bb
                embTsel[:, r] = emb[b, t - 1]

    return {
        "ctxT": ctxT.astype(bf16),
        "wihT": np.ascontiguousarray(Wih.T).astype(bf16),
        "biasf": biasf,
        "whhT": np.ascontiguousarray(Whh.T).astype(bf16),
        "wqT": np.ascontiguousarray(Wq.T).astype(bf16),
        "encT": encT.astype(bf16),
        "vT": vT.astype(bf16),
        "bqT": bqT,
        "maskneg": maskneg,
        "h0T": h0T,
        "bhhnT": bhhnT,
        "alpha": alpha,
        "woutT": np.ascontiguousarray(Wout.T),
        "wembT": np.ascontiguousarray(Wemb.T),
        "embTsel": embTsel,
        "o0T": o0T,
        "wgenT": np.ascontiguousarray(Wgen.T).astype(bf16),
    }


def assemble_output(core_outs):
    """core_outs: list of 8 arrays [ROWS, V] -> full [B, T, V]."""
    res = np.empty((B, T, V), np.float32)
    for core in range(N_CORES):
        st, g = core // 4, core % 4
        ts = _t_list(st)
        o = core_outs[core].reshape(NSTEP, BL, V)
        for ti, t in enumerate(ts):
            res[g * BL:(g + 1) * BL, t, :] = o[ti]
    return res


_PROG = None


def _get_prog():
    global _PROG
    if _PROG is None:
        _PROG = build_program(N_CORES)
    return _PROG


def kernel(**inputs) -> np.ndarray:
    nc = _get_prog()
    in_maps = [prep_core_inputs(c, inputs) for c in range(N_CORES)]
    res = bass_utils.run_bass_kernel_spmd(nc, in_maps, core_ids=list(range(N_CORES)))
    return assemble_output([res.results[c]["out"] for c in range(N_CORES)])

